# revision 13
# baseline (speedup 1.0000x reference)
"""Trainium2 Bass kernel for single-token multi-head self-attention.

Problem (hardcoded):
  q: (1, 32, 512) f32, k/v: (8192, 32, 512) f32, 8 heads x 64 dim,
  scores = (q.k)/8, softcapped 10*tanh(.), softmax over klen, out = w.v.

Strategy: data-parallel over batch, 4 batches per core on 8 cores. The
problem is HBM-bandwidth bound, so K/V/q are staged to device HBM as fp16
(half the traffic; scores only need ~1e-2 accuracy headroom and the final
softmax renormalizes common-mode error away). Per core, K/V stream in
j-chunks of J_FOLD*128 rows (fp16 SBUF tiles (128, J_FOLD*2048)):
  - scores via DVE: k_t *= q_broadcast (fp16 2x mode), two fp16 tree-halving
    adds over d (2x mode), then fp32 segmented reduce of the remaining 16
  - softcap+exp on ACT: e = exp(10*tanh(scores/8)) -> fp16 (no max pass
    needed: scores are clipped to +-10 so exp(s) <= 2.2e4 fits fp16/fp32)
  - P@V and sum(e) accumulated on PE into fp32 PSUM across all chunks
    (lhsT = e-slice (128,8), rhs = v-slice (128,512); ones column for the
    softmax denominator)
Epilogue ships the raw PV block (8, 4x512) and exp-sums (32,) to DRAM in
fp32; the tiny diagonal extraction out[b,h,:] = pv[h, b, h*64:] / s[b,h]
is done on the host (64 KB per core, negligible).
"""

import numpy as np

import concourse.bass as bass
import concourse.bacc as bacc
import concourse.tile as tile
from concourse import mybir
from concourse.bass_utils import run_bass_kernel_spmd

N_CORES = 8
KLEN = 8192
BSZ = 32
D_MODEL = 512
N_HEAD = 8
D_HEAD = 64
B_PER_CORE = BSZ // N_CORES            # 4
BH = B_PER_CORE * N_HEAD               # 32
FREE = B_PER_CORE * D_MODEL            # 2048
P = 128                                # j rows per sub-chunk (partition dim)
J_FOLD = 2                             # sub-chunks folded per DMA/iteration
SCALE = 1.0 / D_HEAD**0.5              # 0.125
CLIP = 10.0

F16 = mybir.dt.float16
F32 = mybir.dt.float32

_PROG_CACHE: dict = {}


def build_program(klen: int = KLEN):
    """Build the per-core Bass program (SPMD: same program, per-core data)."""
    rows = P * J_FOLD
    n_iter = klen // rows
    assert klen % rows == 0

    # Bacc (not plain Bass): its compile() pass splits multi-semaphore waits
    # into event-semaphore chains — TRN2 allows at most 1 wait per instruction.
    nc = bacc.Bacc()
    q_d = nc.dram_tensor("q", [1, FREE], F16, kind="ExternalInput")
    k_d = nc.dram_tensor("k", [klen, B_PER_CORE, D_MODEL], F16, kind="ExternalInput")
    v_d = nc.dram_tensor("v", [klen, B_PER_CORE, D_MODEL], F16, kind="ExternalInput")
    pv_d = nc.dram_tensor(
        "pv", [N_HEAD, B_PER_CORE, D_MODEL], F32, kind="ExternalOutput"
    )
    s_d = nc.dram_tensor("s", [BH, 1], F32, kind="ExternalOutput")

    with tile.TileContext(nc) as tc:
        with (
            tc.tile_pool(name="kv", bufs=6) as kv_pool,
            tc.tile_pool(name="small", bufs=3) as small_pool,
            tc.tile_pool(name="singles", bufs=1) as singles,
            tc.tile_pool(name="psum", bufs=1, space="PSUM") as psum_pool,
        ):
            # q replicated to all 128 partitions via broadcast DMA (SWDGE),
            # then fold-tiled on-chip (cheaper than broadcasting 2x from HBM)
            q_sb = singles.tile([P, J_FOLD, FREE], F16)
            q_ap = q_d[:]
            q_bcast = bass.AP(
                tensor=q_ap.tensor,
                offset=q_ap.offset,
                ap=[[0, P], list(q_ap.ap[-1])],
            )
            nc.gpsimd.dma_start(out=q_sb[:, 0, :], in_=q_bcast)
            for o in range(1, J_FOLD):
                nc.vector.tensor_copy(out=q_sb[:, o, :], in_=q_sb[:, 0, :])

            ones_sb = singles.tile([P, 1], F16)
            nc.vector.memset(ones_sb[:], 1.0)

            # persistent PSUM accumulators
            pv_ps = [
                psum_pool.tile([N_HEAD, D_MODEL], F32, name=f"pv{b}")
                for b in range(B_PER_CORE)
            ]
            s_ps = psum_pool.tile([BH, 1], F32, name="s")

            kv_flat = k_d[:].rearrange("j b d -> j (b d)")
            vv_flat = v_d[:].rearrange("j b d -> j (b d)")

            # fold-2 blocks for the bulk; single-P blocks at the end so the
            # serial tail compute after the last DMA is as small as possible
            blocks = []
            j0 = 0
            while klen - j0 > 2 * P:
                blocks.append((j0, J_FOLD))
                j0 += J_FOLD * P
            while j0 < klen:
                blocks.append((j0, 1))
                j0 += P

            for bi, (j0, fold) in enumerate(blocks):
                k_t = kv_pool.tile([P, fold, FREE], F16, tag="k")
                v_t = kv_pool.tile([P, fold, FREE], F16, tag="v")
                k_src = kv_flat[j0 : j0 + fold * P].rearrange(
                    "(o p) f -> p o f", p=P
                )
                v_src = vv_flat[j0 : j0 + fold * P].rearrange(
                    "(o p) f -> p o f", p=P
                )
                # K on the SP HWDGE ring, V on the ACT HWDGE ring — the two
                # physical rings run concurrently, hiding per-DMA ramp
                nc.sync.dma_start(out=k_t[:], in_=k_src)
                nc.scalar.dma_start(out=v_t[:], in_=v_src)

                # scores: k_t *= q (in place, fp16 2x mode)
                nc.vector.tensor_mul(
                    out=k_t[:], in0=k_t[:], in1=q_sb[:, 0:fold, :]
                )
                # tree-halving partial sums over d (fp16, 2x mode):
                # (p, o, g, 64) -> 32 -> 16 -> 8
                k4 = k_t[:].rearrange("p o (g d) -> p o g d", d=D_HEAD)
                nc.vector.tensor_add(
                    out=k4[:, :, :, 0:32], in0=k4[:, :, :, 0:32], in1=k4[:, :, :, 32:64]
                )
                nc.vector.tensor_add(
                    out=k4[:, :, :, 0:16], in0=k4[:, :, :, 0:16], in1=k4[:, :, :, 16:32]
                )
                nc.vector.tensor_add(
                    out=k4[:, :, :, 0:8], in0=k4[:, :, :, 0:8], in1=k4[:, :, :, 8:16]
                )
                # final fp32 segmented reduce of 8 -> scores (p, o*BH)
                sc = small_pool.tile([P, fold * BH], F32, tag="sc")
                nc.vector.reduce_sum(
                    out=sc[:],
                    in_=k4[:, :, :, 0:8],
                    axis=mybir.AxisListType.X,
                )
                # e = exp(CLIP * tanh(SCALE * raw_score)), fp16 for the PE
                nc.scalar.activation(
                    out=sc[:], in_=sc[:],
                    func=mybir.ActivationFunctionType.Tanh, scale=SCALE,
                )
                e = small_pool.tile([P, fold * BH], F16, tag="e")
                nc.scalar.activation(
                    out=e[:], in_=sc[:],
                    func=mybir.ActivationFunctionType.Exp, scale=CLIP,
                )

                start = bi == 0
                stop = bi == len(blocks) - 1
                for o in range(fold):
                    for b in range(B_PER_CORE):
                        nc.tensor.matmul(
                            pv_ps[b][:],
                            lhsT=e[:, o * BH + b * N_HEAD : o * BH + (b + 1) * N_HEAD],
                            rhs=v_t[:, o, b * D_MODEL : (b + 1) * D_MODEL],
                            start=start and o == 0,
                            stop=stop and o == fold - 1,
                        )
                    nc.tensor.matmul(
                        s_ps[:],
                        lhsT=e[:, o * BH : (o + 1) * BH],
                        rhs=ones_sb[:],
                        start=start and o == 0,
                        stop=stop and o == fold - 1,
                    )

            # epilogue: PSUM -> SBUF -> DRAM (fp32); copies split over ACT+DVE
            pv_sb = singles.tile([N_HEAD, B_PER_CORE * D_MODEL], F32)
            for b in range(B_PER_CORE):
                eng = nc.scalar if b % 2 == 0 else nc.vector
                out_slice = pv_sb[:, b * D_MODEL : (b + 1) * D_MODEL]
                if eng is nc.scalar:
                    nc.scalar.copy(out=out_slice, in_=pv_ps[b][:])
                else:
                    nc.vector.tensor_copy(out=out_slice, in_=pv_ps[b][:])
            s_sb = singles.tile([BH, 1], F32)
            nc.vector.tensor_copy(out=s_sb[:], in_=s_ps[:])
            nc.sync.dma_start(
                out=pv_d[:].rearrange("h b d -> h (b d)"), in_=pv_sb[:]
            )
            nc.sync.dma_start(out=s_d[:], in_=s_sb[:])
    nc.finalize()
    return nc


def shard_inputs(q: np.ndarray, k: np.ndarray, v: np.ndarray):
    """Split full inputs into per-core input maps (fp16 staging)."""
    q16 = np.asarray(q, dtype=np.float16)
    k16 = np.asarray(k, dtype=np.float16)
    v16 = np.asarray(v, dtype=np.float16)
    in_maps = []
    for i in range(N_CORES):
        b0 = i * B_PER_CORE
        in_maps.append(
            {
                "q": np.ascontiguousarray(
                    q16[0, b0 : b0 + B_PER_CORE, :]
                ).reshape(1, FREE),
                "k": np.ascontiguousarray(k16[:, b0 : b0 + B_PER_CORE, :]),
                "v": np.ascontiguousarray(v16[:, b0 : b0 + B_PER_CORE, :]),
            }
        )
    return in_maps


def combine_outputs(results) -> np.ndarray:
    """Per-core (pv, s) -> full (1, 32, 512): diagonal extract + normalize."""
    outs = []
    hh = np.arange(N_HEAD)
    for i in range(N_CORES):
        pv = np.asarray(results[i]["pv"], dtype=np.float32)
        s = np.asarray(results[i]["s"], dtype=np.float32).reshape(
            B_PER_CORE, N_HEAD
        )
        pv4 = pv.reshape(N_HEAD, B_PER_CORE, N_HEAD, D_HEAD)
        diag = pv4[hh, :, hh, :]          # (n_head, b, d_head), row h = head h
        o = diag.transpose(1, 0, 2)       # (b, h, d)
        o = o / s[:, :, None]
        outs.append(o.reshape(B_PER_CORE, D_MODEL))
    return np.concatenate(outs, axis=0)[None, :, :].astype(np.float32)


def kernel(q, k, v):
    q = np.asarray(q, dtype=np.float32)
    k = np.asarray(k, dtype=np.float32)
    v = np.asarray(v, dtype=np.float32)
    assert q.shape == (1, BSZ, D_MODEL) and k.shape == (KLEN, BSZ, D_MODEL)

    if "prog" not in _PROG_CACHE:
        _PROG_CACHE["prog"] = build_program(KLEN)
    nc = _PROG_CACHE["prog"]

    in_maps = shard_inputs(q, k, v)
    res = run_bass_kernel_spmd(nc, in_maps, list(range(N_CORES))).results
    return combine_outputs(res)


if __name__ == "__main__":
    rng = np.random.default_rng(0)
    q = rng.standard_normal((1, BSZ, D_MODEL), dtype=np.float32)
    k = rng.standard_normal((KLEN, BSZ, D_MODEL), dtype=np.float32)
    v = rng.standard_normal((KLEN, BSZ, D_MODEL), dtype=np.float32)
    out = kernel(q, k, v)
    print(out.shape, out.dtype)


# revision 15
# speedup vs baseline: 1.0208x; 1.0208x over previous
"""Trainium2 Bass kernel for single-token multi-head self-attention.

Problem (hardcoded):
  q: (1, 32, 512) f32, k/v: (8192, 32, 512) f32, 8 heads x 64 dim,
  scores = (q.k)/8, softcapped 10*tanh(.), softmax over klen, out = w.v.

Strategy: data-parallel over batch, 4 batches per core on 8 cores. The
problem is HBM-bandwidth bound, so K/V/q are staged to device HBM as fp16
(half the traffic; scores only need ~1e-2 accuracy headroom and the final
softmax renormalizes common-mode error away). Per core, K/V stream in
j-chunks of J_FOLD*128 rows (fp16 SBUF tiles (128, J_FOLD*2048)):
  - scores via DVE: k_t *= q_broadcast (fp16 2x mode), two fp16 tree-halving
    adds over d (2x mode), then fp32 segmented reduce of the remaining 16
  - softcap+exp on ACT: e = exp(10*tanh(scores/8)) -> fp16 (no max pass
    needed: scores are clipped to +-10 so exp(s) <= 2.2e4 fits fp16/fp32)
  - P@V and sum(e) accumulated on PE into fp32 PSUM across all chunks
    (lhsT = e-slice (128,8), rhs = v-slice (128,512); ones column for the
    softmax denominator)
Epilogue ships the raw PV block (8, 4x512) and exp-sums (32,) to DRAM in
fp32; the tiny diagonal extraction out[b,h,:] = pv[h, b, h*64:] / s[b,h]
is done on the host (64 KB per core, negligible).
"""

import numpy as np

import concourse.bass as bass
import concourse.bacc as bacc
import concourse.tile as tile
from concourse import mybir
from concourse.bass_utils import run_bass_kernel_spmd

N_CORES = 8
KLEN = 8192
BSZ = 32
D_MODEL = 512
N_HEAD = 8
D_HEAD = 64
B_PER_CORE = BSZ // N_CORES            # 4
BH = B_PER_CORE * N_HEAD               # 32
FREE = B_PER_CORE * D_MODEL            # 2048
P = 128                                # j rows per sub-chunk (partition dim)
J_FOLD = 2                             # sub-chunks folded per DMA/iteration
SCALE = 1.0 / D_HEAD**0.5              # 0.125
CLIP = 10.0

F16 = mybir.dt.float16
F32 = mybir.dt.float32

# which HWDGE ring carries the V stream: "scalar" (ACT ring, concurrent with
# K's SP ring) or "sync" (same SP ring as K)
V_RING = "scalar"

_PROG_CACHE: dict = {}


def build_program(klen: int = KLEN):
    """Build the per-core Bass program (SPMD: same program, per-core data)."""
    rows = P * J_FOLD
    n_iter = klen // rows
    assert klen % rows == 0

    # Bacc (not plain Bass): its compile() pass splits multi-semaphore waits
    # into event-semaphore chains — TRN2 allows at most 1 wait per instruction.
    nc = bacc.Bacc()
    q_d = nc.dram_tensor("q", [1, FREE], F16, kind="ExternalInput")
    k_d = nc.dram_tensor("k", [klen, B_PER_CORE, D_MODEL], F16, kind="ExternalInput")
    v_d = nc.dram_tensor("v", [klen, B_PER_CORE, D_MODEL], F16, kind="ExternalInput")
    pv_d = nc.dram_tensor(
        "pv", [N_HEAD, B_PER_CORE, D_MODEL], F32, kind="ExternalOutput"
    )
    s_d = nc.dram_tensor("s", [BH, 1], F32, kind="ExternalOutput")

    with tile.TileContext(nc) as tc:
        with (
            tc.tile_pool(name="kv", bufs=6) as kv_pool,
            tc.tile_pool(name="small", bufs=3) as small_pool,
            tc.tile_pool(name="singles", bufs=1) as singles,
            tc.tile_pool(name="psum", bufs=1, space="PSUM") as psum_pool,
        ):
            # q replicated to all 128 partitions via broadcast DMA (SWDGE),
            # then fold-tiled on-chip (cheaper than broadcasting 2x from HBM)
            q_sb = singles.tile([P, J_FOLD, FREE], F16)
            q_ap = q_d[:]
            q_bcast = bass.AP(
                tensor=q_ap.tensor,
                offset=q_ap.offset,
                ap=[[0, P], list(q_ap.ap[-1])],
            )
            nc.gpsimd.dma_start(out=q_sb[:, 0, :], in_=q_bcast)
            for o in range(1, J_FOLD):
                nc.vector.tensor_copy(out=q_sb[:, o, :], in_=q_sb[:, 0, :])

            ones_sb = singles.tile([P, 1], F16)
            nc.vector.memset(ones_sb[:], 1.0)

            # persistent PSUM accumulators
            pv_ps = [
                psum_pool.tile([N_HEAD, D_MODEL], F32, name=f"pv{b}")
                for b in range(B_PER_CORE)
            ]
            s_ps = psum_pool.tile([BH, 1], F32, name="s")

            kv_flat = k_d[:].rearrange("j b d -> j (b d)")
            vv_flat = v_d[:].rearrange("j b d -> j (b d)")

            # fold-2 blocks for the bulk; single-P blocks at the end so the
            # serial tail compute after the last DMA is as small as possible
            blocks = []
            j0 = 0
            while klen - j0 > 2 * P:
                blocks.append((j0, J_FOLD))
                j0 += J_FOLD * P
            while j0 < klen:
                blocks.append((j0, 1))
                j0 += P

            for bi, (j0, fold) in enumerate(blocks):
                k_t = kv_pool.tile([P, fold, FREE], F16, tag="k")
                v_t = kv_pool.tile([P, fold, FREE], F16, tag="v")
                k_src = kv_flat[j0 : j0 + fold * P].rearrange(
                    "(o p) f -> p o f", p=P
                )
                v_src = vv_flat[j0 : j0 + fold * P].rearrange(
                    "(o p) f -> p o f", p=P
                )
                # K on the SP HWDGE ring, V on the ACT HWDGE ring — the two
                # physical rings run concurrently, hiding per-DMA ramp
                nc.sync.dma_start(out=k_t[:], in_=k_src)
                v_eng = nc.scalar if V_RING == "scalar" else nc.sync
                v_eng.dma_start(out=v_t[:], in_=v_src)

                # scores: k_t *= q (in place, fp16 2x mode)
                nc.vector.tensor_mul(
                    out=k_t[:], in0=k_t[:], in1=q_sb[:, 0:fold, :]
                )
                # tree-halving partial sums over d (fp16, 2x mode):
                # (p, o, g, 64) -> 32 -> 16 -> 8
                k4 = k_t[:].rearrange("p o (g d) -> p o g d", d=D_HEAD)
                nc.vector.tensor_add(
                    out=k4[:, :, :, 0:32], in0=k4[:, :, :, 0:32], in1=k4[:, :, :, 32:64]
                )
                nc.vector.tensor_add(
                    out=k4[:, :, :, 0:16], in0=k4[:, :, :, 0:16], in1=k4[:, :, :, 16:32]
                )
                nc.vector.tensor_add(
                    out=k4[:, :, :, 0:8], in0=k4[:, :, :, 0:8], in1=k4[:, :, :, 8:16]
                )
                # final fp32 segmented reduce of 8 -> scores (p, o*BH)
                sc = small_pool.tile([P, fold * BH], F32, tag="sc")
                nc.vector.reduce_sum(
                    out=sc[:],
                    in_=k4[:, :, :, 0:8],
                    axis=mybir.AxisListType.X,
                )
                # e = exp(CLIP * tanh(SCALE * raw_score)), fp16 for the PE
                nc.scalar.activation(
                    out=sc[:], in_=sc[:],
                    func=mybir.ActivationFunctionType.Tanh, scale=SCALE,
                )
                e = small_pool.tile([P, fold * BH], F16, tag="e")
                nc.scalar.activation(
                    out=e[:], in_=sc[:],
                    func=mybir.ActivationFunctionType.Exp, scale=CLIP,
                )

                start = bi == 0
                stop = bi == len(blocks) - 1
                for o in range(fold):
                    for b in range(B_PER_CORE):
                        nc.tensor.matmul(
                            pv_ps[b][:],
                            lhsT=e[:, o * BH + b * N_HEAD : o * BH + (b + 1) * N_HEAD],
                            rhs=v_t[:, o, b * D_MODEL : (b + 1) * D_MODEL],
                            start=start and o == 0,
                            stop=stop and o == fold - 1,
                        )
                    nc.tensor.matmul(
                        s_ps[:],
                        lhsT=e[:, o * BH : (o + 1) * BH],
                        rhs=ones_sb[:],
                        start=start and o == 0,
                        stop=stop and o == fold - 1,
                    )

            # epilogue: PSUM -> SBUF -> DRAM (fp32); copies split over ACT+DVE
            pv_sb = singles.tile([N_HEAD, B_PER_CORE * D_MODEL], F32)
            for b in range(B_PER_CORE):
                eng = nc.scalar if b % 2 == 0 else nc.vector
                out_slice = pv_sb[:, b * D_MODEL : (b + 1) * D_MODEL]
                if eng is nc.scalar:
                    nc.scalar.copy(out=out_slice, in_=pv_ps[b][:])
                else:
                    nc.vector.tensor_copy(out=out_slice, in_=pv_ps[b][:])
            s_sb = singles.tile([BH, 1], F32)
            nc.vector.tensor_copy(out=s_sb[:], in_=s_ps[:])
            nc.sync.dma_start(
                out=pv_d[:].rearrange("h b d -> h (b d)"), in_=pv_sb[:]
            )
            nc.sync.dma_start(out=s_d[:], in_=s_sb[:])
    nc.finalize()
    return nc


def shard_inputs(q: np.ndarray, k: np.ndarray, v: np.ndarray):
    """Split full inputs into per-core input maps (fp16 staging)."""
    q16 = np.asarray(q, dtype=np.float16)
    k16 = np.asarray(k, dtype=np.float16)
    v16 = np.asarray(v, dtype=np.float16)
    in_maps = []
    for i in range(N_CORES):
        b0 = i * B_PER_CORE
        in_maps.append(
            {
                "q": np.ascontiguousarray(
                    q16[0, b0 : b0 + B_PER_CORE, :]
                ).reshape(1, FREE),
                "k": np.ascontiguousarray(k16[:, b0 : b0 + B_PER_CORE, :]),
                "v": np.ascontiguousarray(v16[:, b0 : b0 + B_PER_CORE, :]),
            }
        )
    return in_maps


def combine_outputs(results) -> np.ndarray:
    """Per-core (pv, s) -> full (1, 32, 512): diagonal extract + normalize."""
    outs = []
    hh = np.arange(N_HEAD)
    for i in range(N_CORES):
        pv = np.asarray(results[i]["pv"], dtype=np.float32)
        s = np.asarray(results[i]["s"], dtype=np.float32).reshape(
            B_PER_CORE, N_HEAD
        )
        pv4 = pv.reshape(N_HEAD, B_PER_CORE, N_HEAD, D_HEAD)
        diag = pv4[hh, :, hh, :]          # (n_head, b, d_head), row h = head h
        o = diag.transpose(1, 0, 2)       # (b, h, d)
        o = o / s[:, :, None]
        outs.append(o.reshape(B_PER_CORE, D_MODEL))
    return np.concatenate(outs, axis=0)[None, :, :].astype(np.float32)


def kernel(q, k, v):
    q = np.asarray(q, dtype=np.float32)
    k = np.asarray(k, dtype=np.float32)
    v = np.asarray(v, dtype=np.float32)
    assert q.shape == (1, BSZ, D_MODEL) and k.shape == (KLEN, BSZ, D_MODEL)

    if "prog" not in _PROG_CACHE:
        _PROG_CACHE["prog"] = build_program(KLEN)
    nc = _PROG_CACHE["prog"]

    in_maps = shard_inputs(q, k, v)
    res = run_bass_kernel_spmd(nc, in_maps, list(range(N_CORES))).results
    return combine_outputs(res)


if __name__ == "__main__":
    rng = np.random.default_rng(0)
    q = rng.standard_normal((1, BSZ, D_MODEL), dtype=np.float32)
    k = rng.standard_normal((KLEN, BSZ, D_MODEL), dtype=np.float32)
    v = rng.standard_normal((KLEN, BSZ, D_MODEL), dtype=np.float32)
    out = kernel(q, k, v)
    print(out.shape, out.dtype)


# revision 16
# speedup vs baseline: 1.1854x; 1.1612x over previous
"""Trainium2 Bass kernel for single-token multi-head self-attention.

Problem (hardcoded):
  q: (1, 32, 512) f32, k/v: (8192, 32, 512) f32, 8 heads x 64 dim,
  scores = (q.k)/8, softcapped 10*tanh(.), softmax over klen, out = w.v.

Strategy: data-parallel over batch, 4 batches per core on 8 cores. The
problem is HBM-bandwidth bound, so K/V/q are staged to device HBM as fp16
(half the traffic; scores only need ~1e-2 accuracy headroom and the final
softmax renormalizes common-mode error away). Per core, K/V stream in
j-chunks of J_FOLD*128 rows (fp16 SBUF tiles (128, J_FOLD*2048)):
  - scores via DVE: k_t *= q_broadcast (fp16 2x mode), two fp16 tree-halving
    adds over d (2x mode), then fp32 segmented reduce of the remaining 16
  - softcap+exp on ACT: e = exp(10*tanh(scores/8)) -> fp16 (no max pass
    needed: scores are clipped to +-10 so exp(s) <= 2.2e4 fits fp16/fp32)
  - P@V and sum(e) accumulated on PE into fp32 PSUM across all chunks
    (lhsT = e-slice (128,8), rhs = v-slice (128,512); ones column for the
    softmax denominator)
Epilogue ships the raw PV block (8, 4x512) and exp-sums (32,) to DRAM in
fp32; the tiny diagonal extraction out[b,h,:] = pv[h, b, h*64:] / s[b,h]
is done on the host (64 KB per core, negligible).
"""

import numpy as np

import concourse.bass as bass
import concourse.bacc as bacc
import concourse.tile as tile
from concourse import mybir
from concourse.bass_utils import run_bass_kernel_spmd

N_CORES = 8
KLEN = 8192
BSZ = 32
D_MODEL = 512
N_HEAD = 8
D_HEAD = 64
B_PER_CORE = BSZ // N_CORES            # 4
BH = B_PER_CORE * N_HEAD               # 32
FREE = B_PER_CORE * D_MODEL            # 2048
P = 128                                # j rows per sub-chunk (partition dim)
J_FOLD = 2                             # sub-chunks folded per DMA/iteration
SCALE = 1.0 / D_HEAD**0.5              # 0.125
CLIP = 10.0

F16 = mybir.dt.float16
F32 = mybir.dt.float32

# which HWDGE ring carries the V stream: "scalar" (ACT ring, concurrent with
# K's SP ring) or "sync" (same SP ring as K)
V_RING = "scalar"

_PROG_CACHE: dict = {}


def build_program(klen: int = KLEN):
    """Build the per-core Bass program (SPMD: same program, per-core data)."""
    rows = P * J_FOLD
    n_iter = klen // rows
    assert klen % rows == 0

    # Bacc (not plain Bass): its compile() pass splits multi-semaphore waits
    # into event-semaphore chains — TRN2 allows at most 1 wait per instruction.
    nc = bacc.Bacc()
    q_d = nc.dram_tensor("q", [1, FREE], F16, kind="ExternalInput")
    k_d = nc.dram_tensor("k", [klen, B_PER_CORE, D_MODEL], F16, kind="ExternalInput")
    v_d = nc.dram_tensor("v", [klen, B_PER_CORE, D_MODEL], F16, kind="ExternalInput")
    pv_d = nc.dram_tensor(
        "pv", [N_HEAD, B_PER_CORE, D_MODEL], F32, kind="ExternalOutput"
    )
    s_d = nc.dram_tensor("s", [BH, 1], F32, kind="ExternalOutput")

    with tile.TileContext(nc) as tc:
        with (
            tc.tile_pool(name="kv", bufs=6) as kv_pool,
            tc.tile_pool(name="small", bufs=3) as small_pool,
            tc.tile_pool(name="singles", bufs=1) as singles,
            tc.tile_pool(name="psum", bufs=1, space="PSUM") as psum_pool,
        ):
            # q replicated to all 128 partitions via broadcast DMA (SWDGE),
            # then fold-tiled on-chip (cheaper than broadcasting 2x from HBM)
            q_sb = singles.tile([P, J_FOLD, FREE], F16)
            q_ap = q_d[:]
            q_bcast = bass.AP(
                tensor=q_ap.tensor,
                offset=q_ap.offset,
                ap=[[0, P], list(q_ap.ap[-1])],
            )
            nc.gpsimd.dma_start(out=q_sb[:, 0, :], in_=q_bcast)
            for o in range(1, J_FOLD):
                nc.vector.tensor_copy(out=q_sb[:, o, :], in_=q_sb[:, 0, :])

            ones_sb = singles.tile([P, 1], F16)
            nc.vector.memset(ones_sb[:], 1.0)

            # persistent PSUM accumulators
            pv_ps = [
                psum_pool.tile([N_HEAD, D_MODEL], F32, name=f"pv{b}")
                for b in range(B_PER_CORE)
            ]
            s_ps = psum_pool.tile([BH, 1], F32, name="s")

            kv_flat = k_d[:].rearrange("j b d -> j (b d)")
            vv_flat = v_d[:].rearrange("j b d -> j (b d)")

            # fold-2 blocks for the bulk; single-P blocks at the end so the
            # serial tail compute after the last DMA is as small as possible
            blocks = []
            j0 = 0
            while klen - j0 > 2 * P:
                blocks.append((j0, J_FOLD))
                j0 += J_FOLD * P
            while j0 < klen:
                blocks.append((j0, 1))
                j0 += P

            for bi, (j0, fold) in enumerate(blocks):
                k_t = kv_pool.tile([P, fold, FREE], F16, tag="k")
                v_t = kv_pool.tile([P, fold, FREE], F16, tag="v")
                k_src = kv_flat[j0 : j0 + fold * P].rearrange(
                    "(o p) f -> p o f", p=P
                )
                v_src = vv_flat[j0 : j0 + fold * P].rearrange(
                    "(o p) f -> p o f", p=P
                )
                # K on the SP HWDGE ring, V on the ACT HWDGE ring — the two
                # physical rings run concurrently, hiding per-DMA ramp
                nc.sync.dma_start(out=k_t[:], in_=k_src)
                v_eng = nc.scalar if V_RING == "scalar" else nc.sync
                v_eng.dma_start(out=v_t[:], in_=v_src)

                # scores: k_t *= q (in place, fp16 2x mode)
                nc.vector.tensor_mul(
                    out=k_t[:], in0=k_t[:], in1=q_sb[:, 0:fold, :]
                )
                # tree-halving partial sums over d (fp16, 2x mode):
                # (p, o, g, 64) -> 32 -> 16 -> 8
                k4 = k_t[:].rearrange("p o (g d) -> p o g d", d=D_HEAD)
                nc.vector.tensor_add(
                    out=k4[:, :, :, 0:32], in0=k4[:, :, :, 0:32], in1=k4[:, :, :, 32:64]
                )
                nc.vector.tensor_add(
                    out=k4[:, :, :, 0:16], in0=k4[:, :, :, 0:16], in1=k4[:, :, :, 16:32]
                )
                nc.vector.tensor_add(
                    out=k4[:, :, :, 0:8], in0=k4[:, :, :, 0:8], in1=k4[:, :, :, 8:16]
                )
                nc.vector.tensor_add(
                    out=k4[:, :, :, 0:4], in0=k4[:, :, :, 0:4], in1=k4[:, :, :, 4:8]
                )
                # final fp32 segmented reduce of 4 -> scores (p, o*BH)
                sc = small_pool.tile([P, fold * BH], F32, tag="sc")
                nc.vector.reduce_sum(
                    out=sc[:],
                    in_=k4[:, :, :, 0:4],
                    axis=mybir.AxisListType.X,
                )
                # e = exp(CLIP * tanh(SCALE * raw_score)), fp16 for the PE
                nc.scalar.activation(
                    out=sc[:], in_=sc[:],
                    func=mybir.ActivationFunctionType.Tanh, scale=SCALE,
                )
                e = small_pool.tile([P, fold * BH], F16, tag="e")
                nc.scalar.activation(
                    out=e[:], in_=sc[:],
                    func=mybir.ActivationFunctionType.Exp, scale=CLIP,
                )

                start = bi == 0
                stop = bi == len(blocks) - 1
                for o in range(fold):
                    for b in range(B_PER_CORE):
                        nc.tensor.matmul(
                            pv_ps[b][:],
                            lhsT=e[:, o * BH + b * N_HEAD : o * BH + (b + 1) * N_HEAD],
                            rhs=v_t[:, o, b * D_MODEL : (b + 1) * D_MODEL],
                            start=start and o == 0,
                            stop=stop and o == fold - 1,
                        )
                    nc.tensor.matmul(
                        s_ps[:],
                        lhsT=e[:, o * BH : (o + 1) * BH],
                        rhs=ones_sb[:],
                        start=start and o == 0,
                        stop=stop and o == fold - 1,
                    )

            # epilogue: PSUM -> SBUF -> DRAM (fp32); copies split over ACT+DVE
            pv_sb = singles.tile([N_HEAD, B_PER_CORE * D_MODEL], F32)
            for b in range(B_PER_CORE):
                eng = nc.scalar if b % 2 == 0 else nc.vector
                out_slice = pv_sb[:, b * D_MODEL : (b + 1) * D_MODEL]
                if eng is nc.scalar:
                    nc.scalar.copy(out=out_slice, in_=pv_ps[b][:])
                else:
                    nc.vector.tensor_copy(out=out_slice, in_=pv_ps[b][:])
            s_sb = singles.tile([BH, 1], F32)
            nc.vector.tensor_copy(out=s_sb[:], in_=s_ps[:])
            nc.sync.dma_start(
                out=pv_d[:].rearrange("h b d -> h (b d)"), in_=pv_sb[:]
            )
            nc.sync.dma_start(out=s_d[:], in_=s_sb[:])
    nc.finalize()
    return nc


def shard_inputs(q: np.ndarray, k: np.ndarray, v: np.ndarray):
    """Split full inputs into per-core input maps (fp16 staging)."""
    q16 = np.asarray(q, dtype=np.float16)
    k16 = np.asarray(k, dtype=np.float16)
    v16 = np.asarray(v, dtype=np.float16)
    in_maps = []
    for i in range(N_CORES):
        b0 = i * B_PER_CORE
        in_maps.append(
            {
                "q": np.ascontiguousarray(
                    q16[0, b0 : b0 + B_PER_CORE, :]
                ).reshape(1, FREE),
                "k": np.ascontiguousarray(k16[:, b0 : b0 + B_PER_CORE, :]),
                "v": np.ascontiguousarray(v16[:, b0 : b0 + B_PER_CORE, :]),
            }
        )
    return in_maps


def combine_outputs(results) -> np.ndarray:
    """Per-core (pv, s) -> full (1, 32, 512): diagonal extract + normalize."""
    outs = []
    hh = np.arange(N_HEAD)
    for i in range(N_CORES):
        pv = np.asarray(results[i]["pv"], dtype=np.float32)
        s = np.asarray(results[i]["s"], dtype=np.float32).reshape(
            B_PER_CORE, N_HEAD
        )
        pv4 = pv.reshape(N_HEAD, B_PER_CORE, N_HEAD, D_HEAD)
        diag = pv4[hh, :, hh, :]          # (n_head, b, d_head), row h = head h
        o = diag.transpose(1, 0, 2)       # (b, h, d)
        o = o / s[:, :, None]
        outs.append(o.reshape(B_PER_CORE, D_MODEL))
    return np.concatenate(outs, axis=0)[None, :, :].astype(np.float32)


def kernel(q, k, v):
    q = np.asarray(q, dtype=np.float32)
    k = np.asarray(k, dtype=np.float32)
    v = np.asarray(v, dtype=np.float32)
    assert q.shape == (1, BSZ, D_MODEL) and k.shape == (KLEN, BSZ, D_MODEL)

    if "prog" not in _PROG_CACHE:
        _PROG_CACHE["prog"] = build_program(KLEN)
    nc = _PROG_CACHE["prog"]

    in_maps = shard_inputs(q, k, v)
    res = run_bass_kernel_spmd(nc, in_maps, list(range(N_CORES))).results
    return combine_outputs(res)


if __name__ == "__main__":
    rng = np.random.default_rng(0)
    q = rng.standard_normal((1, BSZ, D_MODEL), dtype=np.float32)
    k = rng.standard_normal((KLEN, BSZ, D_MODEL), dtype=np.float32)
    v = rng.standard_normal((KLEN, BSZ, D_MODEL), dtype=np.float32)
    out = kernel(q, k, v)
    print(out.shape, out.dtype)
